# revision 11
# baseline (speedup 1.0000x reference)
"""GCN (2-layer, GCNConv-style with self-loops + symmetric normalization)
on 8 Trainium2 NeuronCores.

Strategy (graph/data parallel, per the sharding hint):
  - Nodes are permuted (degree-sorted, striped across cores) and sharded:
    core c owns padded rows [c*NPC_PAD, (c+1)*NPC_PAD).
  - Each core computes z1' = dinv * (x @ W1) for its nodes (PE matmuls,
    fp16), then an AllGather builds the full node-feature table in HBM.
  - Edges live on the core owning their destination. The halo exchange is
    the AllGather; per destination-block of 128 nodes the core gathers
    source rows with dma_gather (<=1024 rows per call — the SWDGE ring is
    1024 descriptors — round-robined over 4 SWDGE queues) and segment-sums
    them on the TensorEngine via identity-matmul PSUM accumulation. The
    GCN norm is folded in: dinv[src] into the table rows, dinv[dst] into
    the PSUM-evacuation scale, and the bias enters as a rank-1
    outer-product matmul (sqrt(deg)[dst] x b) in the same PSUM group.
  - dma_gather indices are int16 (table rows < 32768), so the 50176-row
    table is addressed through NV=3 overlapping 32768-row windows; the
    host waterfills each destination's edges across the windows
    (earliest-deadline-first) to minimize per-block slot padding.
  - Layer 2 repeats the pattern with z2' = dinv * (h1 @ W2) (table rows
    padded to 256B, but only the first 96B fetched per descriptor),
    reusing the same gather indices, and ends with a fused log_softmax.
"""
import sys

sys.path.insert(0, "/opt/trn_rl_repo")

import numpy as np

import concourse.bass as bass
import concourse.bacc as bacc
import concourse.tile as tile
import concourse.mybir as mybir
from concourse import bass_utils

P = 128
NC = 8
F16 = mybir.dt.float16
F32 = mybir.dt.float32
I16 = mybir.dt.int16
MAX_CALL_SLOTS = 8          # 1024 rows = HW SWDGE descriptor-ring capacity


class Cfg:
    def __init__(self, N, F_IN, F_HID, F_OUT, view_rows=32768, slot_cap=112,
                 phase=4, single_core=False, nv=3, repeat=1,
                 shared_tables=False):
        self.phase = phase
        self.single_core = single_core
        self.repeat = repeat
        self.shared_tables = shared_tables
        self.N = N
        self.F_IN = F_IN
        self.F_HID = F_HID
        self.F_OUT = F_OUT
        self.FO_PAD = F_OUT + 1          # one pad col for log_softmax masking
        self.KC = (F_IN + P - 1) // P
        self.K_PAD = self.KC * P
        self.NB = (N + NC * P - 1) // (NC * P)
        self.NPC_PAD = self.NB * P
        self.TOT = NC * self.NPC_PAD
        self.VIEW_ROWS = view_rows
        self.NV = 1 if self.TOT <= view_rows else nv
        if self.NV == 1:
            self.BASES = [0]
        else:
            self.BASES = [round(v * (self.TOT - view_rows) / (self.NV - 1))
                          for v in range(self.NV)]
        assert self.BASES[-1] + view_rows >= self.TOT
        self.SLOT_CAP = slot_cap


def _preprocess(x, W1, b1, W2, b2, edge_index, cfg):
    N, NB, NPC_PAD, TOT = cfg.N, cfg.NB, cfg.NPC_PAD, cfg.TOT
    NV, V, BASES = cfg.NV, cfg.VIEW_ROWS, cfg.BASES
    src = np.asarray(edge_index[0], dtype=np.int64)
    dst = np.asarray(edge_index[1], dtype=np.int64)

    indeg = np.bincount(dst, minlength=N)
    deg = (indeg + 1).astype(np.float64)
    dinv = (1.0 / np.sqrt(deg)).astype(np.float32)
    rdinv = np.sqrt(deg).astype(np.float32)

    # ---- node permutation: degree-sorted desc, striped over cores.
    # One "hole" (guaranteed-zero row) is reserved inside every view window
    # to serve as gather padding.
    hole_cores = sorted({min((BASES[v] + V // 2) // NPC_PAD, NC - 1)
                         for v in range(NV)})
    hole_ranks = sorted(((NB - 1) * NC + c) * P + 127 for c in hole_cores)
    assert TOT - len(hole_ranks) >= N
    order = np.argsort(-deg, kind="stable")
    ii = np.arange(N)
    for h in hole_ranks:
        ii = np.where(ii >= h, ii + 1, ii)
    gi, pi = ii // P, ii % P
    assert gi.max() // NC < NB
    pos_of = np.empty(N, dtype=np.int64)
    pos_of[order] = (gi % NC) * NPC_PAD + (gi // NC) * P + pi
    hole_rows = [c * NPC_PAD + NPC_PAD - 1 for c in hole_cores]
    pad_row = []
    for v in range(NV):
        cands = [r for r in hole_rows if BASES[v] <= r < BASES[v] + V]
        assert cands, (v, BASES, hole_rows)
        pad_row.append(cands[0])

    # ---- edge stream incl. self-loops, sorted by destination position
    ps = np.concatenate([pos_of[src], pos_of])
    pd = np.concatenate([pos_of[dst], pos_of])
    eo = np.argsort(pd, kind="stable")
    ps, pd = ps[eo], pd[eo]
    E2 = len(ps)

    # ---- view windows: waterfill each dst's edges across views
    ends = np.array([b + V for b in BASES])
    vhi = np.searchsorted(np.array(BASES), ps, side="right") - 1
    vlo = np.searchsorted(ends, ps, side="right")
    d_tot = np.bincount(pd, minlength=TOT)

    assigned = np.full(E2, -1, dtype=np.int8)
    remaining = d_tot.astype(np.int64).copy()
    n_view = np.zeros((NV, TOT), dtype=np.int64)
    for v in range(NV):
        un = assigned < 0
        must = un & (vhi == v)
        may = un & (vlo <= v) & (vhi > v)
        cnt_must = np.bincount(pd[must], minlength=TOT)
        cnt_may = np.bincount(pd[may], minlength=TOT)
        T = -(-remaining // (NV - v))       # ceil
        take_may = np.clip(T - cnt_must, 0, cnt_may)
        # rank may-edges within dst, earliest-deadline (vhi asc) first
        mi = np.flatnonzero(may)
        mo = mi[np.lexsort((vhi[mi], pd[mi]))]
        pdm = pd[mo]
        st = np.flatnonzero(np.r_[True, pdm[1:] != pdm[:-1]])
        mrank = np.arange(len(mo)) - np.repeat(st, np.diff(np.r_[st, len(mo)]))
        sel = mo[mrank < take_may[pdm]]
        assigned[must] = v
        assigned[sel] = v
        n_view[v] = cnt_must + np.minimum(take_may, cnt_may)
        remaining -= n_view[v]
    assert (assigned >= 0).all() and (remaining == 0).all()

    # ---- per-block scheduled slots (shared across cores)
    S = np.zeros((NV, NB), dtype=np.int64)
    for v in range(NV):
        S[v] = n_view[v].reshape(TOT // P, P).max(axis=1).reshape(NC, NB).max(0)
    oV = np.zeros((NV, NB + 1), dtype=np.int64)
    for v in range(NV):
        oV[v, 1:] = np.cumsum(S[v])
    TV = oV[:, -1]

    # ---- index arrays
    arrs = []
    for v in range(NV):
        arr = np.full((NC, int(TV[v]) * P), pad_row[v] - BASES[v],
                      dtype=np.int32)
        ei = np.flatnonzero(assigned == v)
        pde = pd[ei]
        st = np.flatnonzero(np.r_[True, pde[1:] != pde[:-1]])
        jj = np.arange(len(ei)) - np.repeat(st, np.diff(np.r_[st, len(ei)]))
        core = pde // NPC_PAD
        blk = (pde % NPC_PAD) // P
        pp = pde % P
        flat = (oV[v, blk] + jj) * P + pp
        arr[core, flat] = ps[ei] - BASES[v]
        assert arr.min() >= 0 and arr.max() < V
        arrs.append(arr)

    # ---- chunk packing
    chunks = []
    cur, tot_s = [], 0
    for b in range(NB):
        s = int(S[:, b].sum())
        if cur and tot_s + s > cfg.SLOT_CAP:
            chunks.append(cur)
            cur, tot_s = [], 0
        cur.append(b)
        tot_s += s
    if cur:
        chunks.append(cur)

    def wrap16(arr):   # stream position q -> [q%16, q//16], replicated 8x
        w = arr.reshape(NC, -1, 16).transpose(0, 2, 1).astype(np.int16)
        return np.tile(w, (1, 8, 1))

    idxs = [wrap16(a) for a in arrs]

    # ---- per-position node data
    xp = np.zeros((TOT, cfg.K_PAD), dtype=np.float16)
    xp[pos_of, : cfg.F_IN] = np.asarray(x, np.float32).astype(np.float16)
    dinv_pos = np.zeros(TOT, dtype=np.float32)
    dinv_pos[pos_of] = dinv
    rdinv_pos = np.zeros(TOT, dtype=np.float32)
    rdinv_pos[pos_of] = rdinv

    W1p = np.zeros((cfg.K_PAD, cfg.F_HID), dtype=np.float16)
    W1p[: cfg.F_IN] = np.asarray(W1, np.float32).astype(np.float16)
    W2p = np.zeros((cfg.F_HID, P), dtype=np.float16)
    W2p[:, : cfg.F_OUT] = np.asarray(W2, np.float32).astype(np.float16)
    b1row = np.asarray(b1, np.float32).astype(np.float16).reshape(1, cfg.F_HID)
    b2row = np.zeros((1, P), dtype=np.float16)
    b2row[0, : cfg.F_OUT] = np.asarray(b2, np.float32).astype(np.float16)
    b2row[0, cfg.F_OUT: cfg.FO_PAD] = -60000.0
    ident = np.eye(P, dtype=np.float16)

    in_maps = []
    for c in range(NC):
        xc = xp[c * NPC_PAD:(c + 1) * NPC_PAD]
        m = {
            "xT": np.ascontiguousarray(xc.T).reshape(cfg.KC, P, NPC_PAD),
            "W1p": W1p.reshape(cfg.KC, P, cfg.F_HID),
            "W2p": W2p,
            "b1row": b1row,
            "b2row": b2row,
            "dinvc": np.ascontiguousarray(
                dinv_pos[c * NPC_PAD:(c + 1) * NPC_PAD].reshape(NB, P).T),
            "rdinvT": rdinv_pos[c * NPC_PAD:(c + 1) * NPC_PAD]
                      .reshape(1, NPC_PAD).astype(np.float16),
            "ident": ident,
        }
        for v in range(NV):
            m[f"idx{v}"] = idxs[v][c]
        in_maps.append(m)

    sched = {
        "S": S.tolist(), "oV": oV.tolist(), "TV": [int(t) for t in TV],
        "chunks": chunks,
    }
    return in_maps, sched, pos_of


def _dma_gather_narrow(gps, out_ap, in_ap, idxs_ap, num_idxs, num_idxs_reg,
                       elem_size, elem_step, queue_num=0):
    """dma_gather without the %256B elem-size restriction (non-transpose,
    DRAM source). The 256B-granularity constraint is on the row stride
    (stride_bytes_256 field), not the payload size. HW-verified (smoke9)."""
    from concourse import ap_utils
    gps._assert_queue_num(queue_num)
    assert idxs_ap.dtype == mybir.dt.int16
    assert in_ap.space == bass.MemorySpace.DRAM
    assert in_ap.dtype == out_ap.dtype
    assert ap_utils.ap_is_contiguous(out_ap.ap[1:])
    assert ap_utils.ap_is_contiguous(idxs_ap.ap[1:])
    assert in_ap.ap[0][0] == elem_step
    stride_bytes = elem_step * mybir.dt.size(in_ap.dtype)
    assert stride_bytes % 256 == 0 and stride_bytes // 256 < 256
    assert in_ap.ap[-1][1] == elem_size
    assert out_ap.ap[-1][1] == elem_size
    assert num_idxs % P == 0
    assert out_ap.ap[0][1] * out_ap.ap[1][1] == num_idxs
    _in_ap = gps.lower_ap_dma(in_ap, for_custom_bir_dma=True)
    _idxs_ap = gps.lower_ap(idxs_ap)
    _out_ap = gps.lower_ap(out_ap)
    return gps.add_instruction(
        mybir.InstDMAGatherAnt(
            name=gps.bass.get_next_instruction_name(),
            ins=[*_in_ap, _idxs_ap, gps.lower_val_access(gps.to_reg(num_idxs_reg))],
            outs=[_out_ap],
            transpose=False, num_idxs=num_idxs, elem_size=elem_size,
            stride_bytes_256=stride_bytes // 256, gen_mode=0,
            single_packet=True, queue_num=queue_num,
            sbuf_tokens_per_rank=0, sbuf_free_dim_per_rank=0,
            sbuf_free_dim_pad_per_rank=0, sbuf_byte_offset=0,
        ))


def _build_program(cfg, sch):
    NB, NPC_PAD, TOT = cfg.NB, cfg.NPC_PAD, cfg.TOT
    FH, KC, NV = cfg.F_HID, cfg.KC, cfg.NV
    S = sch["S"]
    oV = sch["oV"]

    nc = bacc.Bacc("TRN2", target_bir_lowering=False, debug=False,
                   num_devices=1 if cfg.single_core else NC,
                   num_swdge_queues=4)
    xT_in = nc.dram_tensor("xT", [KC, P, NPC_PAD], F16, kind="ExternalInput")
    W1_in = nc.dram_tensor("W1p", [KC, P, FH], F16, kind="ExternalInput")
    W2_in = nc.dram_tensor("W2p", [FH, P], F16, kind="ExternalInput")
    b1_in = nc.dram_tensor("b1row", [1, FH], F16, kind="ExternalInput")
    b2_in = nc.dram_tensor("b2row", [1, P], F16, kind="ExternalInput")
    dinv_in = nc.dram_tensor("dinvc", [P, NB], F32, kind="ExternalInput")
    rdinv_in = nc.dram_tensor("rdinvT", [1, NPC_PAD], F16, kind="ExternalInput")
    idx_in = [nc.dram_tensor(f"idx{v}", [P, sch["TV"][v] * P // 16], I16,
                             kind="ExternalInput") for v in range(NV)]
    id_in = nc.dram_tensor("ident", [P, P], F16, kind="ExternalInput")
    out_dram = nc.dram_tensor("out", [NPC_PAD, cfg.FO_PAD], F32,
                              kind="ExternalOutput")

    rg = [list(range(NC))]

    with tile.TileContext(nc) as tc:
        with tc.tile_pool(name="sb", bufs=1) as sb, \
             tc.tile_pool(name="ps", bufs=1, space="PSUM") as ps, \
             tc.tile_pool(name="dram", bufs=1, space="DRAM") as dram:

            # --- constant loads -------------------------------------------
            xT_t, W1_t, xT_free = [], [], []
            for k in range(KC):
                if cfg.repeat == 1:
                    xk, xfree = tc.tile([P, NPC_PAD], F16, name=f"xT_t{k}")
                    xT_free.append(xfree)
                else:
                    xk = sb.tile([P, NPC_PAD], F16, name=f"xT_t{k}")
                nc.sync.dma_start(xk[:], xT_in.ap()[k])
                xT_t.append(xk)
                wk = sb.tile([P, FH], F16, name=f"W1_t{k}")
                nc.sync.dma_start(wk[:], W1_in.ap()[k])
                W1_t.append(wk)
            W2_t = sb.tile([FH, P], F16, name="W2_t")
            nc.sync.dma_start(W2_t[:], W2_in.ap())
            b1_t = sb.tile([1, FH], F16, name="b1_t")
            nc.sync.dma_start(b1_t[:], b1_in.ap())
            b2_t = sb.tile([1, P], F16, name="b2_t")
            nc.sync.dma_start(b2_t[:], b2_in.ap())
            dinv_t = sb.tile([P, NB], F32, name="dinv_t")
            nc.sync.dma_start(dinv_t[:], dinv_in.ap())
            rdinv_t = sb.tile([1, NPC_PAD], F16, name="rdinv_t")
            nc.sync.dma_start(rdinv_t[:], rdinv_in.ap())
            idx_t = []
            for v in range(NV):
                it = sb.tile([P, sch["TV"][v] * P // 16], I16, name=f"idx_t{v}")
                nc.sync.dma_start(it[:], idx_in[v].ap())
                idx_t.append(it)
            id_t = sb.tile([P, P], F16, name="id_t")
            nc.sync.dma_start(id_t[:], id_in.ap())

            aspace = "Shared" if cfg.shared_tables else "Local"
            bounce1 = dram.tile([NPC_PAD, FH], F16, name="bounce1")
            table1 = dram.tile([TOT, FH], F16, name="table1",
                               addr_space=aspace)
            bounce2 = dram.tile([NPC_PAD, P], F16, name="bounce2")
            table2 = dram.tile([TOT, P], F16, name="table2",
                               addr_space=aspace)

            z1_all = sb.tile([P, NB, FH], F16, name="z1_all")
            if cfg.phase >= 2:
                h1_t = sb.tile([P, NB, FH], F16, name="h1_t")
            if cfg.phase >= 3:
                z2_all = sb.tile([P, NB, P], F16, name="z2_all")
            if cfg.phase >= 4:
                out_all = sb.tile([P, NB, cfg.FO_PAD], F32, name="out_all")
            if cfg.phase <= 3:
                dbg = sb.tile([P, cfg.FO_PAD], F32, name="dbgout")
                nc.gpsimd.memset(dbg[:], 0.0)

            qctr = [0]
            for _rep in range(cfg.repeat):
                _emit_body(cfg, sch, nc, sb, ps, qctr,
                           xT_t, W1_t, W2_t, b1_t, b2_t, dinv_t, rdinv_t,
                           idx_t, id_t, bounce1, table1, bounce2, table2,
                           z1_all,
                           h1_t if cfg.phase >= 2 else None,
                           z2_all if cfg.phase >= 3 else None,
                           out_all if cfg.phase >= 4 else None,
                           dbg if cfg.phase <= 3 else None,
                           out_dram, rg,
                           xT_free if cfg.repeat == 1 else [])

    nc.compile()
    return nc


def _emit_body(cfg, sch, nc, sb, ps, qctr, xT_t, W1_t, W2_t, b1_t, b2_t,
               dinv_t, rdinv_t, idx_t, id_t, bounce1, table1, bounce2,
               table2, z1_all, h1_t, z2_all, out_all, dbg, out_dram, rg,
               xT_free):
    NB, NPC_PAD, TOT = cfg.NB, cfg.NPC_PAD, cfg.TOT
    FH, KC, NV = cfg.F_HID, cfg.KC, cfg.NV
    S = sch["S"]
    oV = sch["oV"]

    if True:
        if True:
            # --- layer-1 dense transform: z1' = dinv * (x @ W1) ----------
            for b in range(NB):
                psz = ps.tile([P, FH], F32, tag="ps", bufs=4, name=f"psz{b}")
                for k in range(KC):
                    nc.tensor.matmul(out=psz[:],
                                     lhsT=xT_t[k][:, b * P:(b + 1) * P],
                                     rhs=W1_t[k][:],
                                     start=(k == 0), stop=(k == KC - 1))
                nc.scalar.activation(z1_all[:, b, :], psz[:],
                                     mybir.ActivationFunctionType.Copy,
                                     bias=0.0, scale=dinv_t[:, b:b + 1])
            nc.sync.dma_start(
                bounce1[:].rearrange("(nb p) f -> p nb f", p=P), z1_all[:])
            for f in reversed(xT_free):
                f()

            if cfg.phase >= 1:
                if cfg.single_core:
                    nc.sync.dma_start(table1[0:NPC_PAD, :], bounce1[:])
                else:
                    nc.gpsimd.collective_compute(
                        "AllGather", mybir.AluOpType.bypass, replica_groups=rg,
                        ins=[bounce1.opt()], outs=[table1.opt()])

            if cfg.phase <= 1:
                for b in range(NB):
                    nc.sync.dma_start(out_dram.ap()[b * P:(b + 1) * P, :], dbg[:])

            def agg_phase(table, fw_row, fw_fetch, brow_t, fo, consume):
                """table rows are [*, fw_row] f16 (256B-multiple stride);
                each descriptor fetches the first fw_fetch cols; reduce fo
                cols per block into PSUM; consume(b, psum) finishes it."""
                views = [table[cfg.BASES[v]:cfg.BASES[v] + cfg.VIEW_ROWS,
                               0:fw_fetch] if NV > 1 else table[:, 0:fw_fetch]
                         for v in range(NV)]

                def gather_stream(g_tile, g_off, view, it, o0, n_slots):
                    sslot = 0
                    while sslot < n_slots:
                        m = min(MAX_CALL_SLOTS, n_slots - sslot)
                        _dma_gather_narrow(
                            nc.gpsimd,
                            out_ap=g_tile[:, g_off + sslot:g_off + sslot + m, :],
                            in_ap=view,
                            idxs_ap=it[:, (o0 + sslot) * 8:(o0 + sslot + m) * 8],
                            num_idxs=m * P, num_idxs_reg=m * P,
                            elem_size=fw_fetch, elem_step=fw_row,
                            queue_num=qctr[0] % 4)
                        qctr[0] += 1
                        sslot += m

                for ci, blocks in enumerate(sch["chunks"]):
                    b0, b1_ = blocks[0], blocks[-1]
                    nS = [oV[v][b1_ + 1] - oV[v][b0] for v in range(NV)]
                    g = sb.tile([P, sum(nS), fw_fetch], F16, tag="g", bufs=2,
                                name=f"g{fw_fetch}_{ci}")
                    go = np.r_[0, np.cumsum(nS)]
                    for v in range(NV):
                        if nS[v]:
                            gather_stream(g, int(go[v]), views[v], idx_t[v],
                                          oV[v][b0], nS[v])
                    for b in blocks:
                        pag = ps.tile([P, fo], F32, tag="ps", bufs=4,
                                      name=f"pag{fw_fetch}_{b}")
                        first = True
                        for v in range(NV):
                            for j in range(S[v][b]):
                                nc.tensor.matmul(
                                    out=pag[:], lhsT=id_t[:],
                                    rhs=g[:, int(go[v]) + oV[v][b] - oV[v][b0]
                                          + j, 0:fo],
                                    start=first, stop=False)
                                first = False
                        nc.tensor.matmul(
                            out=pag[:], lhsT=rdinv_t[:, b * P:(b + 1) * P],
                            rhs=brow_t[:, 0:fo], start=first, stop=True)
                        consume(b, pag)

            # --- layer-1 aggregation -> h1 -------------------------------
            if cfg.phase >= 2:

                def l1_consume(b, pag):
                    nc.scalar.activation(h1_t[:, b, :], pag[:],
                                         mybir.ActivationFunctionType.Relu,
                                         bias=0.0, scale=dinv_t[:, b:b + 1])

                agg_phase(table1, FH, FH, b1_t, FH, l1_consume)

            if cfg.phase == 2:
                for b in range(NB):
                    o_t = sb.tile([P, cfg.FO_PAD], F32, tag="o", bufs=3,
                                  name=f"dbg{b}")
                    nc.vector.tensor_copy(o_t[:], h1_t[:, b, 0:cfg.FO_PAD])
                    nc.sync.dma_start(out_dram.ap()[b * P:(b + 1) * P, :], o_t[:])

            # --- layer-2 dense transform: z2' = dinv * (h1 @ W2) ---------
            for b in range(NB if cfg.phase >= 3 else 0):
                pst = ps.tile([P, P], F16, tag="pst", bufs=2, name=f"pst{b}")
                nc.tensor.transpose(out=pst[:], in_=h1_t[:, b, :],
                                    identity=id_t[:])
                h1T = sb.tile([P, P], F16, tag="h1T", bufs=3, name=f"h1T{b}")
                nc.scalar.activation(h1T[:], pst[:],
                                     mybir.ActivationFunctionType.Copy)
                psz2 = ps.tile([P, P], F32, tag="ps", bufs=4, name=f"psz2{b}")
                nc.tensor.matmul(out=psz2[:], lhsT=h1T[:], rhs=W2_t[:],
                                 start=True, stop=True)
                nc.scalar.activation(z2_all[:, b, :], psz2[:],
                                     mybir.ActivationFunctionType.Copy,
                                     bias=0.0, scale=dinv_t[:, b:b + 1])

            if cfg.phase >= 3:
                nc.sync.dma_start(
                    bounce2[:].rearrange("(nb p) f -> p nb f", p=P), z2_all[:])
                if cfg.single_core:
                    nc.sync.dma_start(table2[0:NPC_PAD, :], bounce2[:])
                else:
                    nc.gpsimd.collective_compute(
                        "AllGather", mybir.AluOpType.bypass, replica_groups=rg,
                        ins=[bounce2.opt()], outs=[table2.opt()])

            if cfg.phase == 3:
                for b in range(NB):
                    nc.sync.dma_start(out_dram.ap()[b * P:(b + 1) * P, :],
                                      dbg[:])

            # --- layer-2 aggregation + log_softmax -----------------------
            def l2_consume(b, pag):
                fo = cfg.FO_PAD
                m0 = sb.tile([P, 1], F32, tag="m0", bufs=3, name=f"m0_{b}")
                nc.vector.tensor_reduce(m0[:], pag[:], mybir.AxisListType.X,
                                        mybir.AluOpType.max)
                mneg = sb.tile([P, 1], F32, tag="mneg", bufs=3, name=f"mn{b}")
                nc.vector.tensor_scalar(mneg[:], m0[:], dinv_t[:, b:b + 1],
                                        -1.0, mybir.AluOpType.mult,
                                        mybir.AluOpType.mult)
                e_t = sb.tile([P, fo], F32, tag="e", bufs=3, name=f"e{b}")
                s_t = sb.tile([P, 1], F32, tag="s", bufs=3, name=f"s{b}")
                nc.scalar.activation(e_t[:], pag[:],
                                     mybir.ActivationFunctionType.Exp,
                                     bias=mneg[:], scale=dinv_t[:, b:b + 1],
                                     accum_out=s_t[:])
                lse = sb.tile([P, 1], F32, tag="lse", bufs=3, name=f"ls{b}")
                nc.scalar.activation(lse[:], s_t[:],
                                     mybir.ActivationFunctionType.Ln)
                c_t = sb.tile([P, 1], F32, tag="c", bufs=3, name=f"c{b}")
                nc.vector.tensor_tensor(out=c_t[:], in0=lse[:], in1=mneg[:],
                                        op=mybir.AluOpType.subtract)
                nc.vector.tensor_scalar(out_all[:, b, :], pag[:],
                                        dinv_t[:, b:b + 1],
                                        c_t[:], mybir.AluOpType.mult,
                                        mybir.AluOpType.subtract)

            if cfg.phase >= 4:
                agg_phase(table2, P, cfg.FO_PAD, b2_t, cfg.FO_PAD, l2_consume)
                nc.sync.dma_start(
                    out_dram.ap().rearrange("(nb p) f -> p nb f", p=P),
                    out_all[:])


LAST_RESULTS = None


def kernel(x, W1, b1, W2, b2, edge_index):
    global LAST_RESULTS
    import os
    import time
    cfg = Cfg(N=50000, F_IN=500, F_HID=128, F_OUT=47,
              phase=int(os.environ.get("GCN_PHASE", "4")))
    in_maps, sched, pos_of = _preprocess(x, W1, b1, W2, b2, edge_index, cfg)
    nc = _build_program(cfg, sched)
    res = None
    for attempt in range(3):
        try:
            res = bass_utils.run_bass_kernel_spmd(
                nc, in_maps, core_ids=list(range(NC)))
            break
        except Exception:
            if attempt == 2:
                raise
            time.sleep(5)
    LAST_RESULTS = res
    alls = np.concatenate([np.asarray(res.results[c]["out"])
                           for c in range(NC)], axis=0)
    return alls[pos_of, : cfg.F_OUT].astype(np.float32)



# revision 21
# speedup vs baseline: 1.3774x; 1.3774x over previous
"""GCN (2-layer, GCNConv-style with self-loops + symmetric normalization)
on 8 Trainium2 NeuronCores.

Strategy (graph/data parallel, per the sharding hint):
  - Nodes are permuted (degree-sorted, striped across cores) and sharded:
    core c owns padded rows [c*NPC_PAD, (c+1)*NPC_PAD).
  - Each core computes z1' = dinv * (x @ W1) for its nodes (PE matmuls,
    fp16), then an AllGather builds the full node-feature table in HBM.
  - Edges live on the core owning their destination. The halo exchange is
    the AllGather; per destination-block of 128 nodes the core gathers
    source rows with dma_gather (<=1024 rows per call — the SWDGE ring is
    1024 descriptors — round-robined over 4 SWDGE queues) and segment-sums
    them on the TensorEngine via identity-matmul PSUM accumulation. The
    GCN norm is folded in: dinv[src] into the table rows, dinv[dst] into
    the PSUM-evacuation scale, and the bias enters as a rank-1
    outer-product matmul (sqrt(deg)[dst] x b) in the same PSUM group.
  - dma_gather indices are int16 (table rows < 32768), so the 50176-row
    table is addressed through NV=3 overlapping 32768-row windows; the
    host waterfills each destination's edges across the windows
    (earliest-deadline-first) to minimize per-block slot padding.
  - Layer 2 repeats the pattern with z2' = dinv * (h1 @ W2) (table rows
    padded to 256B, but only the first 96B fetched per descriptor),
    reusing the same gather indices, and ends with a fused log_softmax.
"""
import sys

sys.path.insert(0, "/opt/trn_rl_repo")

import numpy as np

import concourse.bass as bass
import concourse.bacc as bacc
import concourse.tile as tile
import concourse.mybir as mybir
from concourse import bass_utils

import os

P = 128
NC = 8
F16 = mybir.dt.float16
F32 = mybir.dt.float32
I16 = mybir.dt.int16
MAX_CALL_SLOTS = int(os.environ.get("GCN_CALL_SLOTS", "8"))
N_QUEUES = int(os.environ.get("GCN_QUEUES", "4"))
SINGLE_PKT = bool(int(os.environ.get("GCN_SINGLEPKT", "1")))
FETCH_OVR = int(os.environ.get("GCN_FETCH", "0"))  # timing probe only


class Cfg:
    def __init__(self, N, F_IN, F_HID, F_OUT, view_rows=32768, slot_cap=112,
                 phase=4, single_core=False, nv=4, repeat=1,
                 shared_tables=False, no_self=True, sched_mode="lp"):
        self.phase = phase
        self.single_core = single_core
        self.repeat = repeat
        self.shared_tables = shared_tables
        self.no_self = no_self
        self.sched_mode = sched_mode
        self.N = N
        self.F_IN = F_IN
        self.F_HID = F_HID
        self.F_OUT = F_OUT
        self.FO_PAD = F_OUT + 1          # one pad col for log_softmax masking
        self.KC = (F_IN + P - 1) // P
        self.K_PAD = self.KC * P
        self.NB = (N + NC * P - 1) // (NC * P)
        self.NPC_PAD = self.NB * P
        self.TOT = NC * self.NPC_PAD
        self.VIEW_ROWS = view_rows
        self.NV = 1 if self.TOT <= view_rows else nv
        if self.NV == 1:
            self.BASES = [0]
        else:
            self.BASES = [round(v * (self.TOT - view_rows) / (self.NV - 1))
                          for v in range(self.NV)]
        assert self.BASES[-1] + view_rows >= self.TOT
        self.SLOT_CAP = slot_cap


def _preprocess(x, W1, b1, W2, b2, edge_index, cfg):
    N, NB, NPC_PAD, TOT = cfg.N, cfg.NB, cfg.NPC_PAD, cfg.TOT
    NV, V, BASES = cfg.NV, cfg.VIEW_ROWS, cfg.BASES
    src = np.asarray(edge_index[0], dtype=np.int64)
    dst = np.asarray(edge_index[1], dtype=np.int64)

    indeg = np.bincount(dst, minlength=N)
    deg = (indeg + 1).astype(np.float64)
    dinv = (1.0 / np.sqrt(deg)).astype(np.float32)
    rdinv = np.sqrt(deg).astype(np.float32)

    # ---- node permutation: degree-sorted desc, striped over cores.
    # One "hole" (guaranteed-zero row) is reserved inside every view window
    # to serve as gather padding.
    hole_cores = sorted({min((BASES[v] + V // 2) // NPC_PAD, NC - 1)
                         for v in range(NV)})
    hole_ranks = sorted(((NB - 1) * NC + c) * P + 127 for c in hole_cores)
    assert TOT - len(hole_ranks) >= N
    order = np.argsort(-deg, kind="stable")
    ii = np.arange(N)
    for h in hole_ranks:
        ii = np.where(ii >= h, ii + 1, ii)
    gi, pi = ii // P, ii % P
    assert gi.max() // NC < NB
    pos_of = np.empty(N, dtype=np.int64)
    pos_of[order] = (gi % NC) * NPC_PAD + (gi // NC) * P + pi
    hole_rows = [c * NPC_PAD + NPC_PAD - 1 for c in hole_cores]
    pad_row = []
    for v in range(NV):
        cands = [r for r in hole_rows if BASES[v] <= r < BASES[v] + V]
        assert cands, (v, BASES, hole_rows)
        pad_row.append(cands[0])

    # ---- edge stream (self-loops handled at PSUM evacuation when no_self),
    # sorted by destination position
    if cfg.no_self:
        ps = pos_of[src].copy()
        pd = pos_of[dst].copy()
    else:
        ps = np.concatenate([pos_of[src], pos_of])
        pd = np.concatenate([pos_of[dst], pos_of])
    eo = np.argsort(pd, kind="stable")
    ps, pd = ps[eo], pd[eo]
    E2 = len(ps)

    ends = np.array([b + V for b in BASES])
    vhi = np.searchsorted(np.array(BASES), ps, side="right") - 1
    vlo = np.searchsorted(ends, ps, side="right")
    assert (vlo <= vhi).all() and vlo.min() >= 0 and vhi.max() < NV
    d_tot = np.bincount(pd, minlength=TOT)
    blk_of = (np.arange(TOT) % NPC_PAD) // P

    if cfg.sched_mode == "lp":
        # ---- per-block optimal (T_0..T_{NV-1}) via the interval-Hall LP:
        # for every contiguous view interval [a,b], sum_{v in [a,b]} T_v >=
        # max_p (edges of node p only eligible within [a,b]).
        keys = vlo.astype(np.int64) * NV + vhi
        cnt = np.zeros((NV * NV, TOT), dtype=np.int32)
        for k in range(NV * NV):
            m = keys == k
            if m.any():
                cnt[k] = np.bincount(pd[m], minlength=TOT)
        S = np.zeros((NV, NB), dtype=np.int64)
        for b in range(NB):
            sel = blk_of == b
            I = {}
            for a in range(NV):
                for bb in range(a, NV):
                    ks = [lo * NV + hi for lo in range(a, NV)
                          for hi in range(lo, bb + 1)]
                    I[(a, bb)] = int(cnt[ks][:, sel].sum(0).max())
            C = I[(0, NV - 1)]
            best, bestT = None, None
            if NV == 3:
                for T1 in range(I[(1, 1)], C + 1):
                    T0 = max(I[(0, 0)], I[(0, 1)] - T1)
                    T2 = max(I[(2, 2)], I[(1, 2)] - T1)
                    if T1 + T2 < I[(1, 2)] or T0 + T1 < I[(0, 1)]:
                        continue
                    s = T0 + T1 + T2
                    if best is None or s < best:
                        best, bestT = s, [T0, T1, T2]
            elif NV == 4:
                for T1 in range(I[(1, 1)], C + 1):
                    for T2 in range(I[(2, 2)], C + 1):
                        if T1 + T2 < I[(1, 2)]:
                            continue
                        T0 = max(I[(0, 0)], I[(0, 1)] - T1,
                                 I[(0, 2)] - T1 - T2)
                        T3 = max(I[(3, 3)], I[(2, 3)] - T2,
                                 I[(1, 3)] - T1 - T2)
                        s = T0 + T1 + T2 + T3
                        if best is None or s < best:
                            best, bestT = s, [T0, T1, T2, T3]
            else:
                raise ValueError(f"lp schedule supports NV in (3,4), {NV=}")
            if best < C:
                bestT[-1] += C - best
            S[:, b] = bestT

        # ---- EDF assignment against the chosen capacities
        assigned = np.full(E2, -1, dtype=np.int8)
        n_view = np.zeros((NV, TOT), dtype=np.int64)
        cap_pos = np.zeros((NV, TOT), dtype=np.int64)
        for v in range(NV):
            cap_pos[v] = S[v][blk_of]
        for v in range(NV):
            un = assigned < 0
            must = un & (vhi == v)
            may = un & (vlo <= v) & (vhi > v)
            cnt_must = np.bincount(pd[must], minlength=TOT)
            assert (cnt_must <= cap_pos[v]).all()
            cnt_may = np.bincount(pd[may], minlength=TOT)
            take_may = np.clip(cap_pos[v] - cnt_must, 0, cnt_may)
            mi = np.flatnonzero(may)
            mo = mi[np.lexsort((vhi[mi], pd[mi]))]
            pdm = pd[mo]
            st = np.flatnonzero(np.r_[True, pdm[1:] != pdm[:-1]])
            mrank = (np.arange(len(mo))
                     - np.repeat(st, np.diff(np.r_[st, len(mo)])))
            sel2 = mo[mrank < take_may[pdm]]
            assigned[must] = v
            assigned[sel2] = v
            n_view[v] = cnt_must + np.minimum(take_may, cnt_may)
        assert (assigned >= 0).all()
    else:
        # ---- waterfill each dst's edges across views (legacy)
        assigned = np.full(E2, -1, dtype=np.int8)
        remaining = d_tot.astype(np.int64).copy()
        n_view = np.zeros((NV, TOT), dtype=np.int64)
        for v in range(NV):
            un = assigned < 0
            must = un & (vhi == v)
            may = un & (vlo <= v) & (vhi > v)
            cnt_must = np.bincount(pd[must], minlength=TOT)
            cnt_may = np.bincount(pd[may], minlength=TOT)
            T = -(-remaining // (NV - v))       # ceil
            take_may = np.clip(T - cnt_must, 0, cnt_may)
            # rank may-edges within dst, earliest-deadline (vhi asc) first
            mi = np.flatnonzero(may)
            mo = mi[np.lexsort((vhi[mi], pd[mi]))]
            pdm = pd[mo]
            st = np.flatnonzero(np.r_[True, pdm[1:] != pdm[:-1]])
            mrank = (np.arange(len(mo))
                     - np.repeat(st, np.diff(np.r_[st, len(mo)])))
            sel2 = mo[mrank < take_may[pdm]]
            assigned[must] = v
            assigned[sel2] = v
            n_view[v] = cnt_must + np.minimum(take_may, cnt_may)
            remaining -= n_view[v]
        assert (assigned >= 0).all() and (remaining == 0).all()
        # per-block scheduled slots (shared across cores)
        S = np.zeros((NV, NB), dtype=np.int64)
        for v in range(NV):
            S[v] = (n_view[v].reshape(TOT // P, P).max(axis=1)
                    .reshape(NC, NB).max(0))
    oV = np.zeros((NV, NB + 1), dtype=np.int64)
    for v in range(NV):
        oV[v, 1:] = np.cumsum(S[v])
    TV = oV[:, -1]

    # ---- index arrays
    arrs = []
    for v in range(NV):
        arr = np.full((NC, int(TV[v]) * P), pad_row[v] - BASES[v],
                      dtype=np.int32)
        ei = np.flatnonzero(assigned == v)
        pde = pd[ei]
        st = np.flatnonzero(np.r_[True, pde[1:] != pde[:-1]])
        jj = np.arange(len(ei)) - np.repeat(st, np.diff(np.r_[st, len(ei)]))
        core = pde // NPC_PAD
        blk = (pde % NPC_PAD) // P
        pp = pde % P
        flat = (oV[v, blk] + jj) * P + pp
        arr[core, flat] = ps[ei] - BASES[v]
        assert arr.min() >= 0 and arr.max() < V
        arrs.append(arr)

    # ---- chunk packing
    chunks = []
    cur, tot_s = [], 0
    for b in range(NB):
        s = int(S[:, b].sum())
        if cur and tot_s + s > cfg.SLOT_CAP:
            chunks.append(cur)
            cur, tot_s = [], 0
        cur.append(b)
        tot_s += s
    if cur:
        chunks.append(cur)

    def wrap16(arr):   # stream position q -> [q%16, q//16], replicated 8x
        w = arr.reshape(NC, -1, 16).transpose(0, 2, 1).astype(np.int16)
        return np.tile(w, (1, 8, 1))

    idxs = [wrap16(a) for a in arrs]

    # ---- per-position node data
    xp = np.zeros((TOT, cfg.K_PAD), dtype=np.float16)
    xp[pos_of, : cfg.F_IN] = np.asarray(x, np.float32).astype(np.float16)
    dinv_pos = np.zeros(TOT, dtype=np.float32)
    dinv_pos[pos_of] = dinv
    rdinv_pos = np.zeros(TOT, dtype=np.float32)
    rdinv_pos[pos_of] = rdinv

    W1p = np.zeros((cfg.K_PAD, cfg.F_HID), dtype=np.float16)
    W1p[: cfg.F_IN] = np.asarray(W1, np.float32).astype(np.float16)
    W2p = np.zeros((cfg.F_HID, P), dtype=np.float16)
    W2p[:, : cfg.F_OUT] = np.asarray(W2, np.float32).astype(np.float16)
    b1row = np.asarray(b1, np.float32).astype(np.float16).reshape(1, cfg.F_HID)
    b2row = np.zeros((1, P), dtype=np.float16)
    b2row[0, : cfg.F_OUT] = np.asarray(b2, np.float32).astype(np.float16)
    b2row[0, cfg.F_OUT: cfg.FO_PAD] = -60000.0
    ident = np.eye(P, dtype=np.float16)

    in_maps = []
    for c in range(NC):
        xc = xp[c * NPC_PAD:(c + 1) * NPC_PAD]
        m = {
            "xT": np.ascontiguousarray(xc.T).reshape(cfg.KC, P, NPC_PAD),
            "W1p": W1p.reshape(cfg.KC, P, cfg.F_HID),
            "W2p": W2p,
            "b1row": b1row,
            "b2row": b2row,
            "dinvc": np.ascontiguousarray(
                dinv_pos[c * NPC_PAD:(c + 1) * NPC_PAD].reshape(NB, P).T),
            "rdinvT": rdinv_pos[c * NPC_PAD:(c + 1) * NPC_PAD]
                      .reshape(1, NPC_PAD).astype(np.float16),
            "ident": ident,
        }
        for v in range(NV):
            m[f"idx{v}"] = idxs[v][c]
        in_maps.append(m)

    sched = {
        "S": S.tolist(), "oV": oV.tolist(), "TV": [int(t) for t in TV],
        "chunks": chunks,
    }
    return in_maps, sched, pos_of


def _dma_gather_narrow(gps, out_ap, in_ap, idxs_ap, num_idxs, num_idxs_reg,
                       elem_size, elem_step, queue_num=0):
    """dma_gather without the %256B elem-size restriction (non-transpose,
    DRAM source). The 256B-granularity constraint is on the row stride
    (stride_bytes_256 field), not the payload size. HW-verified (smoke9)."""
    from concourse import ap_utils
    gps._assert_queue_num(queue_num)
    assert idxs_ap.dtype == mybir.dt.int16
    assert in_ap.space == bass.MemorySpace.DRAM
    assert in_ap.dtype == out_ap.dtype
    assert ap_utils.ap_is_contiguous(out_ap.ap[1:])
    assert ap_utils.ap_is_contiguous(idxs_ap.ap[1:])
    assert in_ap.ap[0][0] == elem_step
    stride_bytes = elem_step * mybir.dt.size(in_ap.dtype)
    assert stride_bytes % 256 == 0 and stride_bytes // 256 < 256
    assert in_ap.ap[-1][1] == elem_size
    assert out_ap.ap[-1][1] == elem_size
    assert num_idxs % P == 0
    assert out_ap.ap[0][1] * out_ap.ap[1][1] == num_idxs
    _in_ap = gps.lower_ap_dma(in_ap, for_custom_bir_dma=True)
    _idxs_ap = gps.lower_ap(idxs_ap)
    _out_ap = gps.lower_ap(out_ap)
    return gps.add_instruction(
        mybir.InstDMAGatherAnt(
            name=gps.bass.get_next_instruction_name(),
            ins=[*_in_ap, _idxs_ap, gps.lower_val_access(gps.to_reg(num_idxs_reg))],
            outs=[_out_ap],
            transpose=False, num_idxs=num_idxs, elem_size=elem_size,
            stride_bytes_256=stride_bytes // 256, gen_mode=0,
            single_packet=SINGLE_PKT, queue_num=queue_num,
            sbuf_tokens_per_rank=0, sbuf_free_dim_per_rank=0,
            sbuf_free_dim_pad_per_rank=0, sbuf_byte_offset=0,
        ))


def _build_program(cfg, sch):
    NB, NPC_PAD, TOT = cfg.NB, cfg.NPC_PAD, cfg.TOT
    FH, KC, NV = cfg.F_HID, cfg.KC, cfg.NV
    S = sch["S"]
    oV = sch["oV"]

    nc = bacc.Bacc("TRN2", target_bir_lowering=False, debug=False,
                   num_devices=1 if cfg.single_core else NC,
                   num_swdge_queues=4)
    xT_in = nc.dram_tensor("xT", [KC, P, NPC_PAD], F16, kind="ExternalInput")
    W1_in = nc.dram_tensor("W1p", [KC, P, FH], F16, kind="ExternalInput")
    W2_in = nc.dram_tensor("W2p", [FH, P], F16, kind="ExternalInput")
    b1_in = nc.dram_tensor("b1row", [1, FH], F16, kind="ExternalInput")
    b2_in = nc.dram_tensor("b2row", [1, P], F16, kind="ExternalInput")
    dinv_in = nc.dram_tensor("dinvc", [P, NB], F32, kind="ExternalInput")
    rdinv_in = nc.dram_tensor("rdinvT", [1, NPC_PAD], F16, kind="ExternalInput")
    idx_in = [nc.dram_tensor(f"idx{v}", [P, sch["TV"][v] * P // 16], I16,
                             kind="ExternalInput") for v in range(NV)]
    id_in = nc.dram_tensor("ident", [P, P], F16, kind="ExternalInput")
    out_dram = nc.dram_tensor("out", [NPC_PAD, cfg.FO_PAD], F32,
                              kind="ExternalOutput")

    rg = [list(range(NC))]

    with tile.TileContext(nc) as tc:
        with tc.tile_pool(name="sb", bufs=1) as sb, \
             tc.tile_pool(name="ps", bufs=1, space="PSUM") as ps, \
             tc.tile_pool(name="dram", bufs=1, space="DRAM") as dram:

            # --- constant loads -------------------------------------------
            xT_t, W1_t, xT_free = [], [], []
            for k in range(KC):
                if cfg.repeat == 1:
                    xk, xfree = tc.tile([P, NPC_PAD], F16, name=f"xT_t{k}")
                    xT_free.append(xfree)
                else:
                    xk = sb.tile([P, NPC_PAD], F16, name=f"xT_t{k}")
                nc.sync.dma_start(xk[:], xT_in.ap()[k])
                xT_t.append(xk)
                wk = sb.tile([P, FH], F16, name=f"W1_t{k}")
                nc.sync.dma_start(wk[:], W1_in.ap()[k])
                W1_t.append(wk)
            W2_t = sb.tile([FH, P], F16, name="W2_t")
            nc.sync.dma_start(W2_t[:], W2_in.ap())
            b1_t = sb.tile([1, FH], F16, name="b1_t")
            nc.sync.dma_start(b1_t[:], b1_in.ap())
            b2_t = sb.tile([1, P], F16, name="b2_t")
            nc.sync.dma_start(b2_t[:], b2_in.ap())
            dinv_t = sb.tile([P, NB], F32, name="dinv_t")
            nc.sync.dma_start(dinv_t[:], dinv_in.ap())
            rdinv_t = sb.tile([1, NPC_PAD], F16, name="rdinv_t")
            nc.sync.dma_start(rdinv_t[:], rdinv_in.ap())
            idx_t = []
            for v in range(NV):
                it = sb.tile([P, sch["TV"][v] * P // 16], I16, name=f"idx_t{v}")
                nc.sync.dma_start(it[:], idx_in[v].ap())
                idx_t.append(it)
            id_t = sb.tile([P, P], F16, name="id_t")
            nc.sync.dma_start(id_t[:], id_in.ap())

            aspace = "Shared" if cfg.shared_tables else "Local"
            bounce1 = dram.tile([NPC_PAD, FH], F16, name="bounce1")
            table1 = dram.tile([TOT, FH], F16, name="table1",
                               addr_space=aspace)
            bounce2 = dram.tile([NPC_PAD, P], F16, name="bounce2")
            table2 = dram.tile([TOT, P], F16, name="table2",
                               addr_space=aspace)

            z1_all = sb.tile([P, NB, FH], F16, name="z1_all")
            if cfg.phase >= 2:
                h1_t = sb.tile([P, NB, FH], F16, name="h1_t")
            if cfg.phase >= 3:
                z2_all = sb.tile([P, NB, P], F16, name="z2_all")
            if cfg.phase >= 4:
                out_all = sb.tile([P, NB, cfg.FO_PAD], F32, name="out_all")
            if cfg.phase <= 3:
                dbg = sb.tile([P, cfg.FO_PAD], F32, name="dbgout")
                nc.gpsimd.memset(dbg[:], 0.0)

            qctr = [0]
            for _rep in range(cfg.repeat):
                _emit_body(cfg, sch, nc, sb, ps, qctr,
                           xT_t, W1_t, W2_t, b1_t, b2_t, dinv_t, rdinv_t,
                           idx_t, id_t, bounce1, table1, bounce2, table2,
                           z1_all,
                           h1_t if cfg.phase >= 2 else None,
                           z2_all if cfg.phase >= 3 else None,
                           out_all if cfg.phase >= 4 else None,
                           dbg if cfg.phase <= 3 else None,
                           out_dram, rg,
                           xT_free if cfg.repeat == 1 else [])

    nc.compile()
    return nc


def _emit_body(cfg, sch, nc, sb, ps, qctr, xT_t, W1_t, W2_t, b1_t, b2_t,
               dinv_t, rdinv_t, idx_t, id_t, bounce1, table1, bounce2,
               table2, z1_all, h1_t, z2_all, out_all, dbg, out_dram, rg,
               xT_free):
    NB, NPC_PAD, TOT = cfg.NB, cfg.NPC_PAD, cfg.TOT
    FH, KC, NV = cfg.F_HID, cfg.KC, cfg.NV
    S = sch["S"]
    oV = sch["oV"]

    if True:
        if True:
            # --- layer-1 dense transform: z1' = dinv * (x @ W1) ----------
            for b in range(NB):
                psz = ps.tile([P, FH], F32, tag="ps", bufs=4, name=f"psz{b}")
                for k in range(KC):
                    nc.tensor.matmul(out=psz[:],
                                     lhsT=xT_t[k][:, b * P:(b + 1) * P],
                                     rhs=W1_t[k][:],
                                     start=(k == 0), stop=(k == KC - 1))
                nc.scalar.activation(z1_all[:, b, :], psz[:],
                                     mybir.ActivationFunctionType.Copy,
                                     bias=0.0, scale=dinv_t[:, b:b + 1])
            nc.sync.dma_start(
                bounce1[:].rearrange("(nb p) f -> p nb f", p=P), z1_all[:])
            for f in reversed(xT_free):
                f()

            if cfg.phase >= 1:
                if cfg.single_core:
                    nc.sync.dma_start(table1[0:NPC_PAD, :], bounce1[:])
                else:
                    nc.gpsimd.collective_compute(
                        "AllGather", mybir.AluOpType.bypass, replica_groups=rg,
                        ins=[bounce1.opt()], outs=[table1.opt()])

            if cfg.phase <= 1:
                for b in range(NB):
                    nc.sync.dma_start(out_dram.ap()[b * P:(b + 1) * P, :], dbg[:])

            def agg_phase(table, fw_row, fw_fetch, brow_t, fo, consume):
                """table rows are [*, fw_row] f16 (256B-multiple stride);
                each descriptor fetches the first fw_fetch cols; reduce fo
                cols per block into PSUM; consume(b, psum) finishes it."""
                fetch = min(FETCH_OVR, fw_fetch) if FETCH_OVR else fw_fetch
                fo = min(fo, fetch)
                views = [table[cfg.BASES[v]:cfg.BASES[v] + cfg.VIEW_ROWS,
                               0:fetch] if NV > 1 else table[:, 0:fetch]
                         for v in range(NV)]

                def gather_stream(g_tile, g_off, view, it, o0, n_slots):
                    sslot = 0
                    while sslot < n_slots:
                        m = min(MAX_CALL_SLOTS, n_slots - sslot)
                        _dma_gather_narrow(
                            nc.gpsimd,
                            out_ap=g_tile[:, g_off + sslot:g_off + sslot + m,
                                          :],
                            in_ap=view,
                            idxs_ap=it[:, (o0 + sslot) * 8:(o0 + sslot + m) * 8],
                            num_idxs=m * P, num_idxs_reg=m * P,
                            elem_size=fetch, elem_step=fw_row,
                            queue_num=qctr[0] % N_QUEUES)
                        qctr[0] += 1
                        sslot += m

                for ci, blocks in enumerate(sch["chunks"]):
                    b0, b1_ = blocks[0], blocks[-1]
                    nS = [oV[v][b1_ + 1] - oV[v][b0] for v in range(NV)]
                    g = sb.tile([P, sum(nS), fetch], F16, tag="g", bufs=2,
                                name=f"g{fw_fetch}_{ci}")
                    go = np.r_[0, np.cumsum(nS)]
                    for v in range(NV):
                        if nS[v]:
                            gather_stream(g, int(go[v]), views[v], idx_t[v],
                                          oV[v][b0], nS[v])
                    for b in blocks:
                        pag = ps.tile([P, fo], F32, tag="ps", bufs=4,
                                      name=f"pag{fw_fetch}_{b}")
                        first = True
                        for v in range(NV):
                            for j in range(S[v][b]):
                                nc.tensor.matmul(
                                    out=pag[:], lhsT=id_t[:],
                                    rhs=g[:, int(go[v]) + oV[v][b] - oV[v][b0]
                                          + j, 0:fo],
                                    start=first, stop=False)
                                first = False
                        nc.tensor.matmul(
                            out=pag[:], lhsT=rdinv_t[:, b * P:(b + 1) * P],
                            rhs=brow_t[:, 0:fo], start=first, stop=True)
                        consume(b, pag, fo)

            # --- layer-1 aggregation -> h1 -------------------------------
            if cfg.phase >= 2:

                def l1_consume(b, pag, fo):
                    src = pag
                    if cfg.no_self:
                        p2 = sb.tile([P, fo], F32, tag="p2", bufs=3,
                                     name=f"p2_{b}")
                        nc.vector.tensor_tensor(
                            out=p2[:], in0=pag[:], in1=z1_all[:, b, 0:fo],
                            op=mybir.AluOpType.add)
                        src = p2
                    nc.scalar.activation(h1_t[:, b, 0:fo], src[:],
                                         mybir.ActivationFunctionType.Relu,
                                         bias=0.0, scale=dinv_t[:, b:b + 1])

                agg_phase(table1, FH, FH, b1_t, FH, l1_consume)

            if cfg.phase == 2:
                for b in range(NB):
                    o_t = sb.tile([P, cfg.FO_PAD], F32, tag="o", bufs=3,
                                  name=f"dbg{b}")
                    nc.vector.tensor_copy(o_t[:], h1_t[:, b, 0:cfg.FO_PAD])
                    nc.sync.dma_start(out_dram.ap()[b * P:(b + 1) * P, :], o_t[:])

            # --- layer-2 dense transform: z2' = dinv * (h1 @ W2) ---------
            for b in range(NB if cfg.phase >= 3 else 0):
                pst = ps.tile([P, P], F16, tag="pst", bufs=2, name=f"pst{b}")
                nc.tensor.transpose(out=pst[:], in_=h1_t[:, b, :],
                                    identity=id_t[:])
                h1T = sb.tile([P, P], F16, tag="h1T", bufs=3, name=f"h1T{b}")
                nc.scalar.activation(h1T[:], pst[:],
                                     mybir.ActivationFunctionType.Copy)
                psz2 = ps.tile([P, P], F32, tag="ps", bufs=4, name=f"psz2{b}")
                nc.tensor.matmul(out=psz2[:], lhsT=h1T[:], rhs=W2_t[:],
                                 start=True, stop=True)
                nc.scalar.activation(z2_all[:, b, :], psz2[:],
                                     mybir.ActivationFunctionType.Copy,
                                     bias=0.0, scale=dinv_t[:, b:b + 1])

            if cfg.phase >= 3:
                nc.sync.dma_start(
                    bounce2[:].rearrange("(nb p) f -> p nb f", p=P), z2_all[:])
                if cfg.single_core:
                    nc.sync.dma_start(table2[0:NPC_PAD, :], bounce2[:])
                else:
                    nc.gpsimd.collective_compute(
                        "AllGather", mybir.AluOpType.bypass, replica_groups=rg,
                        ins=[bounce2.opt()], outs=[table2.opt()])

            if cfg.phase == 3:
                for b in range(NB):
                    nc.sync.dma_start(out_dram.ap()[b * P:(b + 1) * P, :],
                                      dbg[:])

            # --- layer-2 aggregation + log_softmax -----------------------
            def l2_consume(b, pag, fo):
                if cfg.no_self:
                    p2b = sb.tile([P, fo], F32, tag="p2", bufs=3,
                                  name=f"p2b_{b}")
                    nc.vector.tensor_tensor(
                        out=p2b[:], in0=pag[:], in1=z2_all[:, b, 0:fo],
                        op=mybir.AluOpType.add)
                    pag = p2b
                m0 = sb.tile([P, 1], F32, tag="m0", bufs=3, name=f"m0_{b}")
                nc.vector.tensor_reduce(m0[:], pag[:], mybir.AxisListType.X,
                                        mybir.AluOpType.max)
                mneg = sb.tile([P, 1], F32, tag="mneg", bufs=3, name=f"mn{b}")
                nc.vector.tensor_scalar(mneg[:], m0[:], dinv_t[:, b:b + 1],
                                        -1.0, mybir.AluOpType.mult,
                                        mybir.AluOpType.mult)
                e_t = sb.tile([P, fo], F32, tag="e", bufs=3, name=f"e{b}")
                s_t = sb.tile([P, 1], F32, tag="s", bufs=3, name=f"s{b}")
                nc.scalar.activation(e_t[:], pag[:],
                                     mybir.ActivationFunctionType.Exp,
                                     bias=mneg[:], scale=dinv_t[:, b:b + 1],
                                     accum_out=s_t[:])
                lse = sb.tile([P, 1], F32, tag="lse", bufs=3, name=f"ls{b}")
                nc.scalar.activation(lse[:], s_t[:],
                                     mybir.ActivationFunctionType.Ln)
                c_t = sb.tile([P, 1], F32, tag="c", bufs=3, name=f"c{b}")
                nc.vector.tensor_tensor(out=c_t[:], in0=lse[:], in1=mneg[:],
                                        op=mybir.AluOpType.subtract)
                nc.vector.tensor_scalar(out_all[:, b, 0:fo], pag[:],
                                        dinv_t[:, b:b + 1],
                                        c_t[:], mybir.AluOpType.mult,
                                        mybir.AluOpType.subtract)

            if cfg.phase >= 4:
                agg_phase(table2, P, cfg.FO_PAD, b2_t, cfg.FO_PAD, l2_consume)
                nc.sync.dma_start(
                    out_dram.ap().rearrange("(nb p) f -> p nb f", p=P),
                    out_all[:])


LAST_RESULTS = None


def kernel(x, W1, b1, W2, b2, edge_index):
    global LAST_RESULTS
    import os
    import time
    cfg = Cfg(N=50000, F_IN=500, F_HID=128, F_OUT=47,
              phase=int(os.environ.get("GCN_PHASE", "4")))
    in_maps, sched, pos_of = _preprocess(x, W1, b1, W2, b2, edge_index, cfg)
    nc = _build_program(cfg, sched)
    res = None
    for attempt in range(3):
        try:
            res = bass_utils.run_bass_kernel_spmd(
                nc, in_maps, core_ids=list(range(NC)))
            break
        except Exception:
            if attempt == 2:
                raise
            time.sleep(5)
    LAST_RESULTS = res
    alls = np.concatenate([np.asarray(res.results[c]["out"])
                           for c in range(NC)], axis=0)
    return alls[pos_of, : cfg.F_OUT].astype(np.float32)



# revision 30
# speedup vs baseline: 3.4120x; 2.4772x over previous
"""GCN (2-layer, GCNConv-style with self-loops + symmetric normalization)
on 8 Trainium2 NeuronCores.

Strategy (graph/data parallel, per the sharding hint):
  - Nodes are permuted (degree-sorted, striped across cores) and sharded:
    core c owns padded rows [c*NPC_PAD, (c+1)*NPC_PAD).
  - Each core computes z1' = dinv * (x @ W1) for its nodes (PE matmuls,
    fp16), then an AllGather builds the full node-feature table in HBM.
  - Edges live on the core owning their destination. The halo exchange is
    the AllGather; per destination-block of 128 nodes the core gathers
    source rows with dma_gather (<=1024 rows per call — the SWDGE ring is
    1024 descriptors — round-robined over 4 SWDGE queues) and segment-sums
    them on the TensorEngine via identity-matmul PSUM accumulation. The
    GCN norm is folded in: dinv[src] into the table rows, dinv[dst] into
    the PSUM-evacuation scale, and the bias enters as a rank-1
    outer-product matmul (sqrt(deg)[dst] x b) in the same PSUM group.
  - dma_gather indices are int16 (table rows < 32768), so the 50176-row
    table is addressed through NV=3 overlapping 32768-row windows; the
    host waterfills each destination's edges across the windows
    (earliest-deadline-first) to minimize per-block slot padding.
  - Layer 2 repeats the pattern with z2' = dinv * (h1 @ W2) (table rows
    padded to 256B, but only the first 96B fetched per descriptor),
    reusing the same gather indices, and ends with a fused log_softmax.
"""
import sys

sys.path.insert(0, "/opt/trn_rl_repo")

import numpy as np

import concourse.bass as bass
import concourse.bacc as bacc
import concourse.tile as tile
import concourse.mybir as mybir
from concourse import bass_utils

import os

P = 128
NC = 8
F16 = mybir.dt.float16
F32 = mybir.dt.float32
I16 = mybir.dt.int16
MAX_CALL_SLOTS = int(os.environ.get("GCN_CALL_SLOTS", "8"))
N_QUEUES = int(os.environ.get("GCN_QUEUES", "4"))
SINGLE_PKT = bool(int(os.environ.get("GCN_SINGLEPKT", "1")))
FETCH_OVR = int(os.environ.get("GCN_FETCH", "0"))  # timing probe only
AGG_MODE = int(os.environ.get("GCN_AGGMODE", "0"))  # 1=gathers only, 2=mm only
G_BUFS = int(os.environ.get("GCN_GBUFS", "2"))
SLOT_CAP_OVR = int(os.environ.get("GCN_SLOT_CAP", "0"))


class Cfg:
    def __init__(self, N, F_IN, F_HID, F_OUT, view_rows=32768, slot_cap=112,
                 phase=4, single_core=False, nv=4, repeat=1,
                 shared_tables=False, no_self=True, sched_mode="lp"):
        self.phase = phase
        self.single_core = single_core
        self.repeat = repeat
        self.shared_tables = shared_tables
        self.no_self = no_self
        self.sched_mode = sched_mode
        self.N = N
        self.F_IN = F_IN
        self.F_HID = F_HID
        self.F_OUT = F_OUT
        self.FO_PAD = F_OUT + 1          # one pad col for log_softmax masking
        self.KC = (F_IN + P - 1) // P
        self.K_PAD = self.KC * P
        self.NB = (N + NC * P - 1) // (NC * P)
        self.NPC_PAD = self.NB * P
        self.TOT = NC * self.NPC_PAD
        self.VIEW_ROWS = view_rows
        self.NV = 1 if self.TOT <= view_rows else nv
        if self.NV == 1:
            self.BASES = [0]
        else:
            self.BASES = [round(v * (self.TOT - view_rows) / (self.NV - 1))
                          for v in range(self.NV)]
        assert self.BASES[-1] + view_rows >= self.TOT
        self.SLOT_CAP = SLOT_CAP_OVR or slot_cap


def _preprocess(x, W1, b1, W2, b2, edge_index, cfg):
    N, NB, NPC_PAD, TOT = cfg.N, cfg.NB, cfg.NPC_PAD, cfg.TOT
    NV, V, BASES = cfg.NV, cfg.VIEW_ROWS, cfg.BASES
    src = np.asarray(edge_index[0], dtype=np.int64)
    dst = np.asarray(edge_index[1], dtype=np.int64)

    indeg = np.bincount(dst, minlength=N)
    deg = (indeg + 1).astype(np.float64)
    dinv = (1.0 / np.sqrt(deg)).astype(np.float32)
    rdinv = np.sqrt(deg).astype(np.float32)

    # ---- node permutation: degree-sorted desc, striped over cores.
    # The TOT-N spare positions become "holes" (guaranteed-zero rows) spread
    # uniformly over the rank space; padding descriptors cycle through them
    # so pad gathers don't serialize on a single HBM bank.
    n_holes = TOT - N
    hole_ranks = np.unique(
        np.round(np.linspace(0, TOT - 1, n_holes)).astype(np.int64))
    extra = np.setdiff1d(np.arange(TOT), hole_ranks)
    hole_ranks = np.sort(np.r_[hole_ranks,
                               extra[:n_holes - len(hole_ranks)]])
    assert len(hole_ranks) == n_holes
    real_ranks = np.setdiff1d(np.arange(TOT), hole_ranks)
    assert len(real_ranks) == N
    order = np.argsort(-deg, kind="stable")

    def rank_to_pos(r):
        gi, pi = r // P, r % P
        return (gi % NC) * NPC_PAD + (gi // NC) * P + pi

    pos_of = np.empty(N, dtype=np.int64)
    pos_of[order] = rank_to_pos(real_ranks)
    hole_pos = np.sort(rank_to_pos(hole_ranks))
    pad_rows = []
    for v in range(NV):
        hv = hole_pos[(hole_pos >= BASES[v]) & (hole_pos < BASES[v] + V)]
        assert len(hv), (v, BASES)
        pad_rows.append(hv)

    # ---- edge stream (self-loops handled at PSUM evacuation when no_self),
    # sorted by destination position
    if cfg.no_self:
        ps = pos_of[src].copy()
        pd = pos_of[dst].copy()
    else:
        ps = np.concatenate([pos_of[src], pos_of])
        pd = np.concatenate([pos_of[dst], pos_of])
    eo = np.argsort(pd, kind="stable")
    ps, pd = ps[eo], pd[eo]
    E2 = len(ps)

    ends = np.array([b + V for b in BASES])
    vhi = np.searchsorted(np.array(BASES), ps, side="right") - 1
    vlo = np.searchsorted(ends, ps, side="right")
    assert (vlo <= vhi).all() and vlo.min() >= 0 and vhi.max() < NV
    d_tot = np.bincount(pd, minlength=TOT)
    blk_of = (np.arange(TOT) % NPC_PAD) // P

    if cfg.sched_mode == "lp":
        # ---- per-block optimal (T_0..T_{NV-1}) via the interval-Hall LP:
        # for every contiguous view interval [a,b], sum_{v in [a,b]} T_v >=
        # max_p (edges of node p only eligible within [a,b]).
        keys = vlo.astype(np.int64) * NV + vhi
        cnt = np.zeros((NV * NV, TOT), dtype=np.int32)
        for k in range(NV * NV):
            m = keys == k
            if m.any():
                cnt[k] = np.bincount(pd[m], minlength=TOT)
        S = np.zeros((NV, NB), dtype=np.int64)
        for b in range(NB):
            sel = blk_of == b
            I = {}
            for a in range(NV):
                for bb in range(a, NV):
                    ks = [lo * NV + hi for lo in range(a, NV)
                          for hi in range(lo, bb + 1)]
                    I[(a, bb)] = int(cnt[ks][:, sel].sum(0).max())
            C = I[(0, NV - 1)]
            best, bestT = None, None
            if NV == 3:
                for T1 in range(I[(1, 1)], C + 1):
                    T0 = max(I[(0, 0)], I[(0, 1)] - T1)
                    T2 = max(I[(2, 2)], I[(1, 2)] - T1)
                    if T1 + T2 < I[(1, 2)] or T0 + T1 < I[(0, 1)]:
                        continue
                    s = T0 + T1 + T2
                    if best is None or s < best:
                        best, bestT = s, [T0, T1, T2]
            elif NV == 4:
                for T1 in range(I[(1, 1)], C + 1):
                    for T2 in range(I[(2, 2)], C + 1):
                        if T1 + T2 < I[(1, 2)]:
                            continue
                        T0 = max(I[(0, 0)], I[(0, 1)] - T1,
                                 I[(0, 2)] - T1 - T2)
                        T3 = max(I[(3, 3)], I[(2, 3)] - T2,
                                 I[(1, 3)] - T1 - T2)
                        s = T0 + T1 + T2 + T3
                        if best is None or s < best:
                            best, bestT = s, [T0, T1, T2, T3]
            else:
                raise ValueError(f"lp schedule supports NV in (3,4), {NV=}")
            if best < C:
                bestT[-1] += C - best
            S[:, b] = bestT

        # ---- EDF assignment against the chosen capacities
        assigned = np.full(E2, -1, dtype=np.int8)
        n_view = np.zeros((NV, TOT), dtype=np.int64)
        cap_pos = np.zeros((NV, TOT), dtype=np.int64)
        for v in range(NV):
            cap_pos[v] = S[v][blk_of]
        for v in range(NV):
            un = assigned < 0
            must = un & (vhi == v)
            may = un & (vlo <= v) & (vhi > v)
            cnt_must = np.bincount(pd[must], minlength=TOT)
            assert (cnt_must <= cap_pos[v]).all()
            cnt_may = np.bincount(pd[may], minlength=TOT)
            take_may = np.clip(cap_pos[v] - cnt_must, 0, cnt_may)
            mi = np.flatnonzero(may)
            mo = mi[np.lexsort((vhi[mi], pd[mi]))]
            pdm = pd[mo]
            st = np.flatnonzero(np.r_[True, pdm[1:] != pdm[:-1]])
            mrank = (np.arange(len(mo))
                     - np.repeat(st, np.diff(np.r_[st, len(mo)])))
            sel2 = mo[mrank < take_may[pdm]]
            assigned[must] = v
            assigned[sel2] = v
            n_view[v] = cnt_must + np.minimum(take_may, cnt_may)
        assert (assigned >= 0).all()
    else:
        # ---- waterfill each dst's edges across views (legacy)
        assigned = np.full(E2, -1, dtype=np.int8)
        remaining = d_tot.astype(np.int64).copy()
        n_view = np.zeros((NV, TOT), dtype=np.int64)
        for v in range(NV):
            un = assigned < 0
            must = un & (vhi == v)
            may = un & (vlo <= v) & (vhi > v)
            cnt_must = np.bincount(pd[must], minlength=TOT)
            cnt_may = np.bincount(pd[may], minlength=TOT)
            T = -(-remaining // (NV - v))       # ceil
            take_may = np.clip(T - cnt_must, 0, cnt_may)
            # rank may-edges within dst, earliest-deadline (vhi asc) first
            mi = np.flatnonzero(may)
            mo = mi[np.lexsort((vhi[mi], pd[mi]))]
            pdm = pd[mo]
            st = np.flatnonzero(np.r_[True, pdm[1:] != pdm[:-1]])
            mrank = (np.arange(len(mo))
                     - np.repeat(st, np.diff(np.r_[st, len(mo)])))
            sel2 = mo[mrank < take_may[pdm]]
            assigned[must] = v
            assigned[sel2] = v
            n_view[v] = cnt_must + np.minimum(take_may, cnt_may)
            remaining -= n_view[v]
        assert (assigned >= 0).all() and (remaining == 0).all()
        # per-block scheduled slots (shared across cores)
        S = np.zeros((NV, NB), dtype=np.int64)
        for v in range(NV):
            S[v] = (n_view[v].reshape(TOT // P, P).max(axis=1)
                    .reshape(NC, NB).max(0))
    oV = np.zeros((NV, NB + 1), dtype=np.int64)
    for v in range(NV):
        oV[v, 1:] = np.cumsum(S[v])
    TV = oV[:, -1]

    # ---- index arrays
    srcsort = bool(int(os.environ.get("GCN_SRCSORT", "1")))
    arrs = []
    for v in range(NV):
        arr = np.resize((pad_rows[v] - BASES[v]).astype(np.int32),
                        (NC, int(TV[v]) * P))
        ei = np.flatnonzero(assigned == v)
        if AGG_MODE == 3:
            ei = ei[:0]       # timing probe: every descriptor hits pad_row
        if srcsort:
            # ascending sources per lane: slot j holds each lane's j-th
            # smallest source, so a slot's 128 descriptors concentrate in a
            # narrow HBM address band (order-statistic banding)
            ei = ei[np.lexsort((ps[ei], pd[ei]))]
        pde = pd[ei]
        st = np.flatnonzero(np.r_[True, pde[1:] != pde[:-1]])
        jj = np.arange(len(ei)) - np.repeat(st, np.diff(np.r_[st, len(ei)]))
        core = pde // NPC_PAD
        blk = (pde % NPC_PAD) // P
        pp = pde % P
        flat = (oV[v, blk] + jj) * P + pp
        arr[core, flat] = ps[ei] - BASES[v]
        assert arr.min() >= 0 and arr.max() < V
        arrs.append(arr)

    # ---- chunk packing
    chunks = []
    cur, tot_s = [], 0
    for b in range(NB):
        s = int(S[:, b].sum())
        if cur and tot_s + s > cfg.SLOT_CAP:
            chunks.append(cur)
            cur, tot_s = [], 0
        cur.append(b)
        tot_s += s
    if cur:
        chunks.append(cur)

    def wrap16(arr):   # stream position q -> [q%16, q//16], replicated 8x
        w = arr.reshape(NC, -1, 16).transpose(0, 2, 1).astype(np.int16)
        return np.tile(w, (1, 8, 1))

    idxs = [wrap16(a) for a in arrs]

    # ---- per-position node data
    xp = np.zeros((TOT, cfg.K_PAD), dtype=np.float16)
    xp[pos_of, : cfg.F_IN] = np.asarray(x, np.float32).astype(np.float16)
    dinv_pos = np.zeros(TOT, dtype=np.float32)
    dinv_pos[pos_of] = dinv
    rdinv_pos = np.zeros(TOT, dtype=np.float32)
    rdinv_pos[pos_of] = rdinv

    W1p = np.zeros((cfg.K_PAD, cfg.F_HID), dtype=np.float16)
    W1p[: cfg.F_IN] = np.asarray(W1, np.float32).astype(np.float16)
    W2p = np.zeros((cfg.F_HID, P), dtype=np.float16)
    W2p[:, : cfg.F_OUT] = np.asarray(W2, np.float32).astype(np.float16)
    b1row = np.asarray(b1, np.float32).astype(np.float16).reshape(1, cfg.F_HID)
    b2row = np.zeros((1, P), dtype=np.float16)
    b2row[0, : cfg.F_OUT] = np.asarray(b2, np.float32).astype(np.float16)
    b2row[0, cfg.F_OUT: cfg.FO_PAD] = -60000.0
    ident = np.eye(P, dtype=np.float16)

    in_maps = []
    for c in range(NC):
        xc = xp[c * NPC_PAD:(c + 1) * NPC_PAD]
        m = {
            "xT": np.ascontiguousarray(xc.T).reshape(cfg.KC, P, NPC_PAD),
            "W1p": W1p.reshape(cfg.KC, P, cfg.F_HID),
            "W2p": W2p,
            "b1row": b1row,
            "b2row": b2row,
            "dinvc": np.ascontiguousarray(
                dinv_pos[c * NPC_PAD:(c + 1) * NPC_PAD].reshape(NB, P).T),
            "rdinvT": rdinv_pos[c * NPC_PAD:(c + 1) * NPC_PAD]
                      .reshape(1, NPC_PAD).astype(np.float16),
            "ident": ident,
        }
        for v in range(NV):
            m[f"idx{v}"] = idxs[v][c]
        in_maps.append(m)

    sched = {
        "S": S.tolist(), "oV": oV.tolist(), "TV": [int(t) for t in TV],
        "chunks": chunks,
    }
    return in_maps, sched, pos_of


def _dma_gather_narrow(gps, out_ap, in_ap, idxs_ap, num_idxs, num_idxs_reg,
                       elem_size, elem_step, queue_num=0):
    """dma_gather without the %256B elem-size restriction (non-transpose,
    DRAM source). The 256B-granularity constraint is on the row stride
    (stride_bytes_256 field), not the payload size. HW-verified (smoke9)."""
    from concourse import ap_utils
    gps._assert_queue_num(queue_num)
    assert idxs_ap.dtype == mybir.dt.int16
    assert in_ap.space == bass.MemorySpace.DRAM
    assert in_ap.dtype == out_ap.dtype
    assert ap_utils.ap_is_contiguous(out_ap.ap[1:])
    assert ap_utils.ap_is_contiguous(idxs_ap.ap[1:])
    assert in_ap.ap[0][0] == elem_step
    stride_bytes = elem_step * mybir.dt.size(in_ap.dtype)
    assert stride_bytes % 256 == 0 and stride_bytes // 256 < 256
    assert in_ap.ap[-1][1] == elem_size
    assert out_ap.ap[-1][1] == elem_size
    assert num_idxs % P == 0
    assert out_ap.ap[0][1] * out_ap.ap[1][1] == num_idxs
    _in_ap = gps.lower_ap_dma(in_ap, for_custom_bir_dma=True)
    _idxs_ap = gps.lower_ap(idxs_ap)
    _out_ap = gps.lower_ap(out_ap)
    return gps.add_instruction(
        mybir.InstDMAGatherAnt(
            name=gps.bass.get_next_instruction_name(),
            ins=[*_in_ap, _idxs_ap, gps.lower_val_access(gps.to_reg(num_idxs_reg))],
            outs=[_out_ap],
            transpose=False, num_idxs=num_idxs, elem_size=elem_size,
            stride_bytes_256=stride_bytes // 256, gen_mode=0,
            single_packet=SINGLE_PKT, queue_num=queue_num,
            sbuf_tokens_per_rank=0, sbuf_free_dim_per_rank=0,
            sbuf_free_dim_pad_per_rank=0, sbuf_byte_offset=0,
        ))


def _build_program(cfg, sch):
    NB, NPC_PAD, TOT = cfg.NB, cfg.NPC_PAD, cfg.TOT
    FH, KC, NV = cfg.F_HID, cfg.KC, cfg.NV
    S = sch["S"]
    oV = sch["oV"]

    nc = bacc.Bacc("TRN2", target_bir_lowering=False, debug=False,
                   num_devices=1 if cfg.single_core else NC,
                   num_swdge_queues=4)
    xT_in = nc.dram_tensor("xT", [KC, P, NPC_PAD], F16, kind="ExternalInput")
    W1_in = nc.dram_tensor("W1p", [KC, P, FH], F16, kind="ExternalInput")
    W2_in = nc.dram_tensor("W2p", [FH, P], F16, kind="ExternalInput")
    b1_in = nc.dram_tensor("b1row", [1, FH], F16, kind="ExternalInput")
    b2_in = nc.dram_tensor("b2row", [1, P], F16, kind="ExternalInput")
    dinv_in = nc.dram_tensor("dinvc", [P, NB], F32, kind="ExternalInput")
    rdinv_in = nc.dram_tensor("rdinvT", [1, NPC_PAD], F16, kind="ExternalInput")
    idx_in = [nc.dram_tensor(f"idx{v}", [P, sch["TV"][v] * P // 16], I16,
                             kind="ExternalInput") for v in range(NV)]
    id_in = nc.dram_tensor("ident", [P, P], F16, kind="ExternalInput")
    out_dram = nc.dram_tensor("out", [NPC_PAD, cfg.FO_PAD], F32,
                              kind="ExternalOutput")

    rg = [list(range(NC))]

    with tile.TileContext(nc) as tc:
        with tc.tile_pool(name="sb", bufs=1) as sb, \
             tc.tile_pool(name="ps", bufs=1, space="PSUM") as ps, \
             tc.tile_pool(name="dram", bufs=1, space="DRAM") as dram:

            # --- constant loads -------------------------------------------
            xT_t, W1_t, xT_free = [], [], []
            for k in range(KC):
                if cfg.repeat == 1:
                    xk, xfree = tc.tile([P, NPC_PAD], F16, name=f"xT_t{k}")
                    xT_free.append(xfree)
                else:
                    xk = sb.tile([P, NPC_PAD], F16, name=f"xT_t{k}")
                nc.sync.dma_start(xk[:], xT_in.ap()[k])
                xT_t.append(xk)
                wk = sb.tile([P, FH], F16, name=f"W1_t{k}")
                nc.sync.dma_start(wk[:], W1_in.ap()[k])
                W1_t.append(wk)
            W2_t = sb.tile([FH, P], F16, name="W2_t")
            nc.sync.dma_start(W2_t[:], W2_in.ap())
            b1_t = sb.tile([1, FH], F16, name="b1_t")
            nc.sync.dma_start(b1_t[:], b1_in.ap())
            b2_t = sb.tile([1, P], F16, name="b2_t")
            nc.sync.dma_start(b2_t[:], b2_in.ap())
            dinv_t = sb.tile([P, NB], F32, name="dinv_t")
            nc.sync.dma_start(dinv_t[:], dinv_in.ap())
            rdinv_t = sb.tile([1, NPC_PAD], F16, name="rdinv_t")
            nc.sync.dma_start(rdinv_t[:], rdinv_in.ap())
            idx_t = []
            for v in range(NV):
                it = sb.tile([P, sch["TV"][v] * P // 16], I16, name=f"idx_t{v}")
                nc.sync.dma_start(it[:], idx_in[v].ap())
                idx_t.append(it)
            id_t = sb.tile([P, P], F16, name="id_t")
            nc.sync.dma_start(id_t[:], id_in.ap())

            aspace = "Shared" if cfg.shared_tables else "Local"
            bounce1 = dram.tile([NPC_PAD, FH], F16, name="bounce1")
            table1 = dram.tile([TOT, FH], F16, name="table1",
                               addr_space=aspace)
            bounce2 = dram.tile([NPC_PAD, P], F16, name="bounce2")
            table2 = dram.tile([TOT, P], F16, name="table2",
                               addr_space=aspace)

            z1_all = sb.tile([P, NB, FH], F16, name="z1_all")
            if cfg.phase >= 2:
                h1_t = sb.tile([P, NB, FH], F16, name="h1_t")
            if cfg.phase >= 3:
                z2_all = sb.tile([P, NB, P], F16, name="z2_all")
            if cfg.phase >= 4:
                out_all = sb.tile([P, NB, cfg.FO_PAD], F32, name="out_all")
            if cfg.phase <= 3:
                dbg = sb.tile([P, cfg.FO_PAD], F32, name="dbgout")
                nc.gpsimd.memset(dbg[:], 0.0)

            qctr = [0]
            for _rep in range(cfg.repeat):
                _emit_body(cfg, sch, nc, sb, ps, qctr,
                           xT_t, W1_t, W2_t, b1_t, b2_t, dinv_t, rdinv_t,
                           idx_t, id_t, bounce1, table1, bounce2, table2,
                           z1_all,
                           h1_t if cfg.phase >= 2 else None,
                           z2_all if cfg.phase >= 3 else None,
                           out_all if cfg.phase >= 4 else None,
                           dbg if cfg.phase <= 3 else None,
                           out_dram, rg,
                           xT_free if cfg.repeat == 1 else [])

    nc.compile()
    return nc


def _emit_body(cfg, sch, nc, sb, ps, qctr, xT_t, W1_t, W2_t, b1_t, b2_t,
               dinv_t, rdinv_t, idx_t, id_t, bounce1, table1, bounce2,
               table2, z1_all, h1_t, z2_all, out_all, dbg, out_dram, rg,
               xT_free):
    NB, NPC_PAD, TOT = cfg.NB, cfg.NPC_PAD, cfg.TOT
    FH, KC, NV = cfg.F_HID, cfg.KC, cfg.NV
    S = sch["S"]
    oV = sch["oV"]

    if True:
        if True:
            # --- layer-1 dense transform: z1' = dinv * (x @ W1) ----------
            for b in range(NB):
                psz = ps.tile([P, FH], F32, tag="ps", bufs=4, name=f"psz{b}")
                for k in range(KC):
                    nc.tensor.matmul(out=psz[:],
                                     lhsT=xT_t[k][:, b * P:(b + 1) * P],
                                     rhs=W1_t[k][:],
                                     start=(k == 0), stop=(k == KC - 1))
                nc.scalar.activation(z1_all[:, b, :], psz[:],
                                     mybir.ActivationFunctionType.Copy,
                                     bias=0.0, scale=dinv_t[:, b:b + 1])
            nc.sync.dma_start(
                bounce1[:].rearrange("(nb p) f -> p nb f", p=P), z1_all[:])
            for f in reversed(xT_free):
                f()

            if cfg.phase >= 1:
                if cfg.single_core:
                    nc.sync.dma_start(table1[0:NPC_PAD, :], bounce1[:])
                else:
                    nc.gpsimd.collective_compute(
                        "AllGather", mybir.AluOpType.bypass, replica_groups=rg,
                        ins=[bounce1.opt()], outs=[table1.opt()])

            if cfg.phase <= 1:
                for b in range(NB):
                    nc.sync.dma_start(out_dram.ap()[b * P:(b + 1) * P, :], dbg[:])

            def agg_phase(table, fw_row, fw_fetch, brow_t, fo, consume):
                """table rows are [*, fw_row] f16 (256B-multiple stride);
                each descriptor fetches the first fw_fetch cols; reduce fo
                cols per block into PSUM; consume(b, psum) finishes it."""
                fetch = min(FETCH_OVR, fw_fetch) if FETCH_OVR else fw_fetch
                fo = min(fo, fetch)
                views = [table[cfg.BASES[v]:cfg.BASES[v] + cfg.VIEW_ROWS,
                               0:fetch] if NV > 1 else table[:, 0:fetch]
                         for v in range(NV)]

                def gather_stream(g_tile, g_off, view, it, o0, n_slots):
                    sslot = 0
                    while sslot < n_slots:
                        m = min(MAX_CALL_SLOTS, n_slots - sslot)
                        _dma_gather_narrow(
                            nc.gpsimd,
                            out_ap=g_tile[:, g_off + sslot:g_off + sslot + m,
                                          :],
                            in_ap=view,
                            idxs_ap=it[:, (o0 + sslot) * 8:(o0 + sslot + m) * 8],
                            num_idxs=m * P, num_idxs_reg=m * P,
                            elem_size=fetch, elem_step=fw_row,
                            queue_num=qctr[0] % N_QUEUES)
                        qctr[0] += 1
                        sslot += m

                for ci, blocks in enumerate(sch["chunks"]):
                    b0, b1_ = blocks[0], blocks[-1]
                    nS = [oV[v][b1_ + 1] - oV[v][b0] for v in range(NV)]
                    g = sb.tile([P, sum(nS), fetch], F16, tag="g",
                                bufs=G_BUFS, name=f"g{fw_fetch}_{ci}")
                    go = np.r_[0, np.cumsum(nS)]
                    if AGG_MODE != 2:
                        for v in range(NV):
                            if nS[v]:
                                gather_stream(g, int(go[v]), views[v],
                                              idx_t[v], oV[v][b0], nS[v])
                    for b in (blocks if AGG_MODE != 1 else []):
                        pag = ps.tile([P, fo], F32, tag="ps", bufs=4,
                                      name=f"pag{fw_fetch}_{b}")
                        first = True
                        for v in range(NV):
                            for j in range(S[v][b]):
                                nc.tensor.matmul(
                                    out=pag[:], lhsT=id_t[:],
                                    rhs=g[:, int(go[v]) + oV[v][b] - oV[v][b0]
                                          + j, 0:fo],
                                    start=first, stop=False)
                                first = False
                        nc.tensor.matmul(
                            out=pag[:], lhsT=rdinv_t[:, b * P:(b + 1) * P],
                            rhs=brow_t[:, 0:fo], start=first, stop=True)
                        consume(b, pag, fo)

            # --- layer-1 aggregation -> h1 -------------------------------
            if cfg.phase >= 2:

                def l1_consume(b, pag, fo):
                    src = pag
                    if cfg.no_self:
                        p2 = sb.tile([P, fo], F32, tag="p2", bufs=3,
                                     name=f"p2_{b}")
                        nc.vector.tensor_tensor(
                            out=p2[:], in0=pag[:], in1=z1_all[:, b, 0:fo],
                            op=mybir.AluOpType.add)
                        src = p2
                    nc.scalar.activation(h1_t[:, b, 0:fo], src[:],
                                         mybir.ActivationFunctionType.Relu,
                                         bias=0.0, scale=dinv_t[:, b:b + 1])

                agg_phase(table1, FH, FH, b1_t, FH, l1_consume)

            if cfg.phase == 2:
                h1_like = z1_all if AGG_MODE == 1 else h1_t
                for b in range(NB):
                    o_t = sb.tile([P, cfg.FO_PAD], F32, tag="o", bufs=3,
                                  name=f"dbg{b}")
                    nc.vector.tensor_copy(o_t[:], h1_like[:, b, 0:cfg.FO_PAD])
                    nc.sync.dma_start(out_dram.ap()[b * P:(b + 1) * P, :], o_t[:])

            # --- layer-2 dense transform: z2' = dinv * (h1 @ W2) ---------
            for b in range(NB if cfg.phase >= 3 else 0):
                pst = ps.tile([P, P], F16, tag="pst", bufs=2, name=f"pst{b}")
                nc.tensor.transpose(out=pst[:], in_=h1_t[:, b, :],
                                    identity=id_t[:])
                h1T = sb.tile([P, P], F16, tag="h1T", bufs=3, name=f"h1T{b}")
                nc.scalar.activation(h1T[:], pst[:],
                                     mybir.ActivationFunctionType.Copy)
                psz2 = ps.tile([P, P], F32, tag="ps", bufs=4, name=f"psz2{b}")
                nc.tensor.matmul(out=psz2[:], lhsT=h1T[:], rhs=W2_t[:],
                                 start=True, stop=True)
                nc.scalar.activation(z2_all[:, b, :], psz2[:],
                                     mybir.ActivationFunctionType.Copy,
                                     bias=0.0, scale=dinv_t[:, b:b + 1])

            if cfg.phase >= 3:
                nc.sync.dma_start(
                    bounce2[:].rearrange("(nb p) f -> p nb f", p=P), z2_all[:])
                if cfg.single_core:
                    nc.sync.dma_start(table2[0:NPC_PAD, :], bounce2[:])
                else:
                    nc.gpsimd.collective_compute(
                        "AllGather", mybir.AluOpType.bypass, replica_groups=rg,
                        ins=[bounce2.opt()], outs=[table2.opt()])

            if cfg.phase == 3:
                for b in range(NB):
                    nc.sync.dma_start(out_dram.ap()[b * P:(b + 1) * P, :],
                                      dbg[:])

            # --- layer-2 aggregation + log_softmax -----------------------
            def l2_consume(b, pag, fo):
                if cfg.no_self:
                    p2b = sb.tile([P, fo], F32, tag="p2", bufs=3,
                                  name=f"p2b_{b}")
                    nc.vector.tensor_tensor(
                        out=p2b[:], in0=pag[:], in1=z2_all[:, b, 0:fo],
                        op=mybir.AluOpType.add)
                    pag = p2b
                m0 = sb.tile([P, 1], F32, tag="m0", bufs=3, name=f"m0_{b}")
                nc.vector.tensor_reduce(m0[:], pag[:], mybir.AxisListType.X,
                                        mybir.AluOpType.max)
                mneg = sb.tile([P, 1], F32, tag="mneg", bufs=3, name=f"mn{b}")
                nc.vector.tensor_scalar(mneg[:], m0[:], dinv_t[:, b:b + 1],
                                        -1.0, mybir.AluOpType.mult,
                                        mybir.AluOpType.mult)
                e_t = sb.tile([P, fo], F32, tag="e", bufs=3, name=f"e{b}")
                s_t = sb.tile([P, 1], F32, tag="s", bufs=3, name=f"s{b}")
                nc.scalar.activation(e_t[:], pag[:],
                                     mybir.ActivationFunctionType.Exp,
                                     bias=mneg[:], scale=dinv_t[:, b:b + 1],
                                     accum_out=s_t[:])
                lse = sb.tile([P, 1], F32, tag="lse", bufs=3, name=f"ls{b}")
                nc.scalar.activation(lse[:], s_t[:],
                                     mybir.ActivationFunctionType.Ln)
                c_t = sb.tile([P, 1], F32, tag="c", bufs=3, name=f"c{b}")
                nc.vector.tensor_tensor(out=c_t[:], in0=lse[:], in1=mneg[:],
                                        op=mybir.AluOpType.subtract)
                nc.vector.tensor_scalar(out_all[:, b, 0:fo], pag[:],
                                        dinv_t[:, b:b + 1],
                                        c_t[:], mybir.AluOpType.mult,
                                        mybir.AluOpType.subtract)

            if cfg.phase >= 4:
                agg_phase(table2, P, cfg.FO_PAD, b2_t, cfg.FO_PAD, l2_consume)
                nc.sync.dma_start(
                    out_dram.ap().rearrange("(nb p) f -> p nb f", p=P),
                    out_all[:])


LAST_RESULTS = None


def kernel(x, W1, b1, W2, b2, edge_index):
    global LAST_RESULTS
    import os
    import time
    cfg = Cfg(N=50000, F_IN=500, F_HID=128, F_OUT=47,
              phase=int(os.environ.get("GCN_PHASE", "4")))
    in_maps, sched, pos_of = _preprocess(x, W1, b1, W2, b2, edge_index, cfg)
    nc = _build_program(cfg, sched)
    res = None
    for attempt in range(3):
        try:
            res = bass_utils.run_bass_kernel_spmd(
                nc, in_maps, core_ids=list(range(NC)))
            break
        except Exception:
            if attempt == 2:
                raise
            time.sleep(5)
    LAST_RESULTS = res
    alls = np.concatenate([np.asarray(res.results[c]["out"])
                           for c in range(NC)], axis=0)
    return alls[pos_of, : cfg.F_OUT].astype(np.float32)

